# revision 21
# baseline (speedup 1.0000x reference)
"""Trainium2 Bass kernel for per-token grouped attention (GQA-style).

Computation (per token t):
    q = x @ Wq.T + bq ; k = x @ Wk.T + bk ; v = x @ Wv.T + bv     (D=2048)
    reshape to (G=16 groups, d=128); scores = q_g . k_h / sqrt(d) (16x16)
    att = softmax(scores, axis=h); out = att @ v  -> (G*d,)

Sharding: data-parallel over the B*T = 16384 tokens across 8 cores
(2048 tokens/core).  Device works feature-major for the projections; the
attention emits the output token-major ([(g,s), block, dd]) and the host
unscrambles.

Device program (per core, SPMD), 4 chunks of 512 tokens:
  Projections: W.T-tiles @ xT with fp32 PSUM accumulation.  Q and K use a
    hybrid contraction: the first 768 input features in bf16, the last
    1280 in fp8-e4m3 with DoubleRow perf mode (2 k-tiles per instruction,
    2x PE throughput) -- the fp8 quantization noise is attenuated through
    the softmax, keeping total rel err ~1.55e-2 (< 2e-2 gate).  V stays
    full bf16 (its error passes straight to the output).  x is scaled by
    16 and W by 8192 on the host so the fp8 operands sit in e4m3's normal
    range; the PSUM->SBUF bias copy (ACT) un-scales by 2^-17.  Q/K's
    bf16+fp8 weight tiles are packed into one uint8 DMA per m-tile pair
    and bitcast on-device (one DMA sem per two PSUM chains).
  Attention per 8-token block b (3 matmuls instead of 4):
    sT = k_blk^T q_blk  (one 128x128 MM: all 64 pairwise 16x16 tiles,
         only the 8 diagonal ones survive the mask)
    e  = exp(sT) * blockdiag-mask          (ACT + DVE)
    vT = PE-transpose(v_blk)               (1 MM)
    o|dn = e^T @ [vT | ones]  (129-col MM: unnormalized out^T[(g,s), dd]
         plus softmax denominators in the 129th column, via a ones
         column pre-set in the vt ring buffers)
    out = o * (1/dn) broadcast             (DVE, batched per half-SB)
  Pump schedule (keeps PE fed, allows single-buffered q/k/v tiles):
    chunk c's q,k slots <- att@v pieces of chunk c-1 (transpose stage
                           runs two pieces ahead of the matmul stage)
    chunk c's v slots   <- scores pieces of chunk c
    after last chunk    <- drain att@v of last chunk
"""

import os
import numpy as np
import ml_dtypes

import concourse.bass as bass
import concourse.tile as tile
from concourse import bacc, mybir
from concourse.bass_utils import run_bass_kernel_spmd

F32 = mybir.dt.float32
BF16 = mybir.dt.bfloat16
FP8 = mybir.dt.float8e4
AF = mybir.ActivationFunctionType
ALU = mybir.AluOpType

P = 128          # SBUF partitions
D = 2048         # model dim
G = 16           # groups
DG = 128         # per-group dim
N_CORES = 8
TC = 2048        # tokens per core
NCHUNK = 4
CH = TC // NCHUNK          # 512 tokens per chunk
NB = CH // 8               # 64 blocks of 8 tokens per chunk
NSB = NB // 4              # 16 super-blocks (32 tokens) per chunk
KT = D // P      # 16 contraction tiles
KA = 6           # bf16 k-tiles (features 0..767)
KB = 10          # fp8 k-tiles (features 768..2047), as KB//2 DoubleRow pairs
WAB = KA * P * 2 + KB * P  # packed qk weight bytes per partition per m-tile
MT = D // P      # 16 output-feature tiles
SCL = 2.0 ** -17  # undo the x*16 / W*8192 scaling in the PSUM->SBUF copy
OTB = 16         # blocks per output tile (128 tokens)


def _emit(nc, tc, ctx):
    # ---- DRAM I/O -------------------------------------------------------
    xTa = nc.dram_tensor("xTa", [NCHUNK, P, KA, CH], BF16,
                         kind="ExternalInput").ap()
    xTb8 = nc.dram_tensor("xTb8", [NCHUNK, P, KB, CH], FP8,
                          kind="ExternalInput").ap()
    xTbb = nc.dram_tensor("xTbb", [NCHUNK, P, KB, CH], BF16,
                          kind="ExternalInput").ap()
    wQK = {
        p: nc.dram_tensor(f"w{p}AB", [MT // 2, P, 2, WAB], mybir.dt.uint8,
                          kind="ExternalInput").ap()
        for p in "qk"
    }
    wV = nc.dram_tensor("wvT", [MT // 2, P, 2, KT, P], BF16,
                        kind="ExternalInput").ap()
    b_dram = nc.dram_tensor("bqkv", [P, 3, G], F32, kind="ExternalInput").ap()
    mi_dram = nc.dram_tensor("m01ident", [P, 2, P], BF16,
                             kind="ExternalInput").ap()
    outT = nc.dram_tensor("outT", [P, TC // 8, P], BF16,
                          kind="ExternalOutput").ap()

    # ---- pools ----------------------------------------------------------
    singles = ctx.enter_context(tc.tile_pool(name="singles", bufs=1))
    xpool = ctx.enter_context(tc.tile_pool(name="xpool", bufs=2))
    wpool = ctx.enter_context(tc.tile_pool(name="wpool", bufs=6))
    wpab = ctx.enter_context(tc.tile_pool(name="wpab", bufs=6))
    asmp = ctx.enter_context(tc.tile_pool(name="asmp", bufs=1))
    epool = ctx.enter_context(tc.tile_pool(name="epool", bufs=NSB + 2))
    vtpool = ctx.enter_context(tc.tile_pool(name="vtpool", bufs=3))
    recp = ctx.enter_context(tc.tile_pool(name="recp", bufs=3))
    otp = ctx.enter_context(tc.tile_pool(name="otp", bufs=2))

    pp_ps = ctx.enter_context(tc.tile_pool(name="pp_ps", bufs=2, space="PSUM"))
    ps_s = ctx.enter_context(tc.tile_pool(name="ps_s", bufs=1, space="PSUM"))
    ps_vt = ctx.enter_context(tc.tile_pool(name="ps_vt", bufs=2, space="PSUM"))
    ps_o = ctx.enter_context(tc.tile_pool(name="ps_o", bufs=3, space="PSUM"))

    # ---- constants (keep the SP/sync queue free for weight tiles) -------
    ball = singles.tile([P, 3, G], F32, tag="bias", name="bias")
    nc.scalar.dma_start(out=ball[:], in_=b_dram[:])
    bias_sb = {p: ball[:, i, :] for i, p in enumerate("qkv")}
    mi_sb = singles.tile([P, 2, P], BF16, tag="mi", name="mi")
    m01_sb = mi_sb[:, 0, :]
    ident_sb = mi_sb[:, 1, :]

    # pre-warm the vt ring: the 129th column stays 1.0 forever (the
    # ones-feature that makes att@v emit softmax denominators in column P)
    for _ in range(3):
        vtw = vtpool.tile([P, 4, P + 1], BF16, tag="vts", name="vts")
        nc.vector.memset(vtw[:, :, P:P + 1], 1.0)

    # per-chunk assembled q/k/v (block-interleaved [dd, block, g, s]),
    # single-buffered: the pump schedule guarantees producer/consumer order.
    asm = {}

    # ---- attention pieces ----------------------------------------------
    st_w = {}       # current quad weight tile (q,k slots)
    st_e = {}       # (sb) -> masked exp tile for current chunk's scores
    st_vt = {}      # (sb) -> transposed-v SBUF tile (stage b_t -> b_m)
    st_ot = {}      # out tile in progress

    def piece_a(c, sb):
        """Scores + exp + mask for super-block sb of chunk c."""
        q2f = asm["q"].rearrange("p b g s -> p (b g s)")
        k2f = asm["k"].rearrange("p b g s -> p (b g s)")
        sT = ps_s.tile([P, 4, P], F32, tag="s", name="s")
        for j in range(4):
            sl = slice((sb * 4 + j) * P, (sb * 4 + j + 1) * P)
            nc.tensor.matmul(sT[:, j, :], lhsT=k2f[:, sl], rhs=q2f[:, sl],
                             start=True, stop=True)
        e = epool.tile([P, 4, P], BF16, tag="e", name="e")
        nc.scalar.activation(out=e[:], in_=sT[:], func=AF.Exp)
        m01_bc = m01_sb.unsqueeze(1).broadcast_to([P, 4, P])
        nc.vector.tensor_tensor(out=e[:], in0=e[:], in1=m01_bc, op=ALU.mult)
        st_e[sb] = e

    def piece_bt(c, sb):
        """Stage 1: v-transpose + PSUM->SBUF copy (with ones column)."""
        v2f = asm["v"].rearrange("p b g s -> p (b g s)")
        vt_ps = ps_vt.tile([P, 4, P], BF16, tag="vt", name="vt")
        for j in range(4):
            sl = slice((sb * 4 + j) * P, (sb * 4 + j + 1) * P)
            nc.tensor.transpose(vt_ps[:, j, :], v2f[:, sl], ident_sb)
        # vt has a 129th column preset to 1.0 (ones-feature -> denominators)
        vt = vtpool.tile([P, 4, P + 1], BF16, tag="vts", name="vts")
        nc.scalar.copy(out=vt[:, :, 0:P], in_=vt_ps[:])
        st_vt[sb] = vt

    def piece_bm(c, sb):
        """Stage 2: att@v (with fused denominators) + normalize + store."""
        if sb % 4 == 0:
            st_ot["t"] = otp.tile([P, OTB, P], BF16, tag="ot", name="ot")
        e = st_e.pop(sb)
        vt = st_vt.pop(sb)
        for h in range(2):
            o_ps = ps_o.tile([P, 2, P + 1], F32, tag="o", name="o")
            for jj in range(2):
                j = h * 2 + jj
                nc.tensor.matmul(o_ps[:, jj, :], lhsT=e[:, j, :],
                                 rhs=vt[:, j, :], start=True, stop=True)
            rec = recp.tile([P, 2, 1], F32, tag="rec", name="rec")
            nc.vector.reciprocal(out=rec[:], in_=o_ps[:, :, P:P + 1])
            rec_bc = rec[:].broadcast_to([P, 2, P])
            base = (sb % 4) * 4 + h * 2
            dst = st_ot["t"][:, base:base + 2, :]
            nc.vector.tensor_tensor(out=dst, in0=o_ps[:, :, 0:P],
                                    in1=rec_bc, op=ALU.mult)
        if sb % 4 == 3:
            t0 = c * NB + (sb - 3) * 4          # first block of this out tile
            nc.gpsimd.dma_start(out=outT[:, t0:t0 + OTB, :], in_=st_ot["t"][:])

    # ---- projections with attention pieces pumped in --------------------
    def load_x(c):
        xa = xpool.tile([P, KA, CH], BF16, tag="xa", name="xa")
        nc.gpsimd.dma_start(out=xa[:], in_=xTa[c])
        xb8 = xpool.tile([P, KB, CH], FP8, tag="xb8", name="xb8")
        nc.gpsimd.dma_start(out=xb8[:], in_=xTb8[c])
        xbb = xpool.tile([P, KB, CH], BF16, tag="xbb", name="xbb")
        nc.gpsimd.dma_start(out=xbb[:], in_=xTbb[c])
        return (xa, xb8, xbb)

    def proj_pair(p, mp, xt):
        """Multiple m-tiles per slot, one weight DMA -> one sem wait."""
        xa, xb8, xbb = xt
        if p == "v":
            w2 = wpool.tile([P, 2, KT, P], BF16, tag="wt", name="wt")
            nc.sync.dma_start(out=w2[:], in_=wV[mp])
        else:
            w2 = wpab.tile([P, 2, WAB], mybir.dt.uint8, tag="wab", name="wab")
            nc.sync.dma_start(out=w2[:], in_=wQK[p][mp])
        for i in range(2):
            m = 2 * mp + i
            ps = pp_ps.tile([P, CH], F32, tag="pp", name="pp")
            if p == "v":
                w = w2[:, i]
                for k in range(KT):
                    src = xa[:, k, :] if k < KA else xbb[:, k - KA, :]
                    nc.tensor.matmul(ps[:], lhsT=w[:, k, :], rhs=src,
                                     start=(k == 0), stop=(k == KT - 1))
            else:
                wa = w2[:, i, 0:KA * P * 2].bitcast(BF16).rearrange(
                    "p (k o) -> p k o", o=P)
                wb = w2[:, i, KA * P * 2:WAB].bitcast(FP8).rearrange(
                    "p (j t o) -> p j t o", t=2, o=P)
                for k in range(KA):
                    nc.tensor.matmul(ps[:], lhsT=wa[:, k, :], rhs=xa[:, k, :],
                                     start=(k == 0), stop=False)
                for j in range(KB // 2):
                    nc.tensor.matmul(ps[:], lhsT=wb[:, j, :, :],
                                     rhs=xb8[:, 2 * j:2 * j + 2, :],
                                     start=False, stop=(j == KB // 2 - 1),
                                     perf_mode=mybir.MatmulPerfMode.DoubleRow)
            dst = asm[p][:, :, m, :]
            src = ps[:].rearrange("p (b s) -> p b s", s=8)
            nc.scalar.activation(out=dst, in_=src, func=AF.Identity,
                                 bias=bias_sb[p][:, m:m + 1], scale=SCL)

    xts = {0: load_x(0)}
    for c in range(NCHUNK):
        xt = xts.pop(c)
        pending_b = []
        if c > 0:
            bt = [lambda sb=sb: piece_bt(c - 1, sb) for sb in range(NSB)]
            bm = [lambda sb=sb: piece_bm(c - 1, sb) for sb in range(NSB)]
            pending_b = [bt[0], bt[1]]
            for sb in range(NSB):
                if sb + 2 < NSB:
                    pending_b.append(bt[sb + 2])
                pending_b.append(bm[sb])
        # q,k slots: pump previous chunk's att@v pieces (2 per slot)
        for i, p in enumerate("qk"):
            asm[p] = asmp.tile([P, NB, G, 8], BF16, tag=f"asm{p}",
                               name=f"asm{p}")
            for mp in range(MT // 2):
                proj_pair(p, mp, xt)
                for _ in range(2):
                    if pending_b:
                        pending_b.pop(0)()
        while pending_b:
            pending_b.pop(0)()
        if c == 0:
            nc.scalar.dma_start(out=mi_sb[:], in_=mi_dram[:])
        # v slots: pump this chunk's scores pieces (1 per slot)
        if c + 1 < NCHUNK:
            xts[c + 1] = load_x(c + 1)
        asm["v"] = asmp.tile([P, NB, G, 8], BF16, tag="asmv", name="asmv")
        for mp in range(MT // 2):
            proj_pair("v", mp, xt)
            piece_a(c, 2 * mp)
            piece_a(c, 2 * mp + 1)
    # drain: att@v of the last chunk, transposes two steps ahead
    piece_bt(NCHUNK - 1, 0)
    piece_bt(NCHUNK - 1, 1)
    for sb in range(NSB):
        if sb + 2 < NSB:
            piece_bt(NCHUNK - 1, sb + 2)
        piece_bm(NCHUNK - 1, sb)


_PROGRAM = None


def _build():
    global _PROGRAM
    if _PROGRAM is not None:
        return _PROGRAM
    from contextlib import ExitStack

    nc = bacc.Bacc("TRN2", target_bir_lowering=False, debug=False,
                   num_devices=N_CORES)
    with tile.TileContext(nc) as tc:
        with ExitStack() as ctx:
            _emit(nc, tc, ctx)
    nc.compile()
    _PROGRAM = nc
    return nc


def _host_inputs(x, Wq, bq, Wk, bk, Wv, bv):
    """Build the per-core input maps (host-side shard + transpose + cast)."""
    scale = 1.0 / np.sqrt(DG)
    xf = np.ascontiguousarray(x.reshape(-1, D))           # [16384, D]
    assert xf.shape[0] == N_CORES * TC

    bf = ml_dtypes.bfloat16
    e4 = ml_dtypes.float8_e4m3

    def tile_w(WT, dtype=bf):
        # [D_in, D_out] -> [MT, P, KT_part, P]: contiguous per m-tile
        kt = WT.shape[0] // P
        a = WT.reshape(kt, P, MT, P).transpose(2, 1, 0, 3)
        return np.ascontiguousarray(a).astype(dtype)

    KAF = KA * P            # bf16-contracted features

    def pack_qk(WT):
        a = tile_w(WT[:KAF] * 8192)                      # [MT,P,KA,P] bf16
        b = tile_w(WT[KAF:] * 8192, e4)                  # [MT,P,KB,P] fp8
        ab = np.concatenate([
            a.view(np.uint8).reshape(MT, P, -1),
            b.view(np.uint8).reshape(MT, P, -1)], axis=2)
        # pair m-tiles: [MT,P,WAB] -> [MT/2, P, 2, WAB]
        ab = ab.reshape(MT // 2, 2, P, WAB).transpose(0, 2, 1, 3)
        return np.ascontiguousarray(ab)

    shared = {
        "wqAB": pack_qk((Wq * scale).T),
        "wkAB": pack_qk(Wk.T),
        "wvT": np.ascontiguousarray(
            tile_w(Wv.T * 8192).reshape(MT // 2, 2, P, KT, P)
            .transpose(0, 2, 1, 3, 4)),
        "bqkv": np.ascontiguousarray(np.stack([
            (bq * scale).reshape(G, DG).T,
            bk.reshape(G, DG).T,
            bv.reshape(G, DG).T], axis=1)).astype(np.float32),
        "m01ident": np.ascontiguousarray(np.stack([
            np.kron(np.ones((G, G), dtype=np.float32),
                    np.eye(8, dtype=np.float32)),
            np.eye(P, dtype=np.float32)], axis=1)).astype(bf),
    }
    in_maps = []
    for i in range(N_CORES):
        xi = xf[i * TC:(i + 1) * TC]
        m = dict(shared)
        # [TC, D] -> tiled [NCHUNK, P, kt, CH], scaled by 16
        xs = (xi.T * 16).reshape(KT, P, NCHUNK, CH).transpose(2, 1, 0, 3)
        m["xTa"] = np.ascontiguousarray(xs[:, :, :KA]).astype(bf)
        m["xTb8"] = np.ascontiguousarray(xs[:, :, KA:]).astype(e4)
        m["xTbb"] = np.ascontiguousarray(xs[:, :, KA:]).astype(bf)
        in_maps.append(m)
    return in_maps


last_results = None


def _install_ntff_shim():
    """Provide antenv.axon_hooks if the image lacks it (profiling only)."""
    import sys
    try:
        from antenv.axon_hooks import get_axon_ntff_profile_hook  # noqa: F401
        return
    except ImportError:
        pass
    import contextlib
    import ctypes
    import types

    so_path = "/opt/axon/libaxon_pjrt.so"
    hook = None
    if os.path.exists(so_path):
        lib = ctypes.CDLL(so_path)
        if hasattr(lib, "axon_start_nrt_profile"):
            lib.axon_start_nrt_profile.argtypes = [
                ctypes.POINTER(ctypes.c_int64), ctypes.c_size_t]
            lib.axon_start_nrt_profile.restype = ctypes.c_int64
            lib.axon_stop_nrt_profile.argtypes = [ctypes.c_char_p]
            lib.axon_stop_nrt_profile.restype = ctypes.c_int64

            @contextlib.contextmanager
            def _hook(output_dir, device_ids):
                import jax
                jax.devices()
                if device_ids:
                    ids = (ctypes.c_int64 * len(device_ids))(*device_ids)
                    rc = lib.axon_start_nrt_profile(ids, len(device_ids))
                else:
                    rc = lib.axon_start_nrt_profile(None, 0)
                if rc != 0:
                    raise RuntimeError(f"axon_start_nrt_profile rc={rc}")
                try:
                    yield
                finally:
                    n = lib.axon_stop_nrt_profile(str(output_dir).encode())
                    print(f"profile: {n} file(s) written to {output_dir}")

            hook = _hook

    mod = types.ModuleType("antenv.axon_hooks")
    mod.get_axon_ntff_profile_hook = lambda: hook
    mod.set_axon_ntff_profile_hook = lambda h: None
    import antenv
    antenv.axon_hooks = mod
    sys.modules["antenv.axon_hooks"] = mod


def kernel(**inputs):
    global last_results
    nc = _build()
    in_maps = _host_inputs(**inputs)
    trace = bool(os.environ.get("BASS_TRACE"))
    if trace:
        _install_ntff_shim()
    res = run_bass_kernel_spmd(nc, in_maps, list(range(N_CORES)), trace=trace)
    last_results = res
    x = inputs["x"]
    out = np.empty((N_CORES * TC, D), dtype=np.float32)
    for i in range(N_CORES):
        o = res.results[i]["outT"].astype(np.float32)      # [P, TC/8, P]
        o = o.reshape(G, 8, TC // 8, DG).transpose(2, 1, 0, 3)
        out[i * TC:(i + 1) * TC] = o.reshape(TC, D)
    return out.reshape(x.shape)
